# revision 19
# baseline (speedup 1.0000x reference)
"""Trainium2 Bass kernel for nn_ConBiMambaBlock (conformer macaron block with a
BiMamba mixer), pure data-parallel over 8 NeuronCores.

Per-core layout (batch shard BB rows, L=3 positions):
  - tokens l-major: token t = l*BB + b, T = 3*BB.
  - residual stream x: SBUF fp32, token-major [128 tok, 512 feat] tiles.
  - matmul land: feature-major fp16 [128 feat, tokens]; weights fp16 [K, M].
  - mamba scan: b-major fp16 [128 tok, 1024 chan]; per-token scalars are
    per-partition scalars.  The L=3 selective scan is unrolled in closed form:
      y_i = (C_i.B_i) dt_i xm_i + D xm_i + sum_{j<i} [sum_n C_i[n]B_j[n] q_ij^(n+1)] dt_j xm_j
    with q_ij = exp(-(dt_{j+1}+..+dt_i)) since A[d,n] = -(n+1) (asserted host-side).
    The 16-term power sums run as Horner chains of scalar_tensor_tensor ops.
  - LN gains/biases, the macaron 0.5, the mamba causal conv (k=4), and BN are all
    folded into weights/biases host-side.  The conformer depthwise conv (k=31)
    touches only taps 13..17 at L=3 and runs as 3-tap per-partition STT chains.
"""

import os
from contextlib import ExitStack

import numpy as np

import concourse.bass as bass
import concourse.mybir as mybir
import concourse.tile as tile
from concourse.masks import make_identity
from concourse.vector_clock import ScopedClock, VectorClock

AF = mybir.ActivationFunctionType
FP32 = mybir.dt.float32
FP16 = mybir.dt.float16
ALU = mybir.AluOpType

D = 512
DI = 1024
DS = 16
RK = 32
H = 2048
L = 3
NCORES = 8
EPS = 1e-5


# ---------------------------------------------------------------------------
# Workaround: this container's walrus rejects >2 sync-wait commands on one
# instruction; Tile's tail drain carries one wait per touched proc.  Split the
# waits across single-proc SP nops (the drain then needs none of its own).
def _patched_drain_and_barrier(self, tick_clock, wait_clock):
    nc = self.nc
    gvec = list(tick_clock.global_clock)
    n = len(gvec)
    for i, t in enumerate(gvec):
        if t <= 0:
            continue
        sub = [0] * n
        sub[i] = t
        nop_inst = nc.sync.nop()
        wait_clock.add_sem_waits(nop_inst.ins, ScopedClock({None: VectorClock(sub)}))
    nc.sync.drain()
    nc.all_engine_barrier()
    popped = nc._tile_sem_poison_stack.pop()
    assert popped is self._sem_poison
    nc.clear_and_free_semaphores(list(self.sems.allocated().values()))
    nc.all_engine_barrier()


tile.TileContext._drain_and_barrier = _patched_drain_and_barrier

MAX_WAITS = 1


def split_excess_waits(nc, maxw=MAX_WAITS):
    """Post-pass: any instruction with more than `maxw` sem waits gets the
    excess hoisted onto freshly inserted same-engine nops just before it
    (engines execute their subsequence in order, so this is equivalent)."""
    nnop = 0
    for f in nc.m.functions:
        for b in f.blocks:
            il = b.instructions
            out = []
            for inst in il:
                si = inst.sync_info
                if si is not None and si.on_wait and len(si.on_wait) > maxw:
                    waits = list(si.on_wait)
                    while len(waits) > maxw:
                        chunk, waits = waits[:maxw], waits[maxw:]
                        nop = mybir.InstNoOp(
                            name=f"I-waitsplit-{nnop}",
                            sync_info=mybir.SyncInfo(on_wait=chunk,
                                                     on_update=[]))
                        nnop += 1
                        nop.engine = inst.engine
                        nc.register_instruction(nop)
                        out.append(nop)
                    si.on_wait = waits
                out.append(inst)
            if nnop:
                b.instructions = out
    return nnop


# ---------------------------------------------------------------------------
def _hilo(b):
    """fp32 vector -> [2, N] fp16 (hi, lo) for exact rank-1 bias matmuls."""
    b = np.asarray(b, np.float32)
    hi = b.astype(np.float16)
    lo = (b - hi.astype(np.float32)).astype(np.float16)
    return np.stack([hi, lo], 0)


def _perpart(v, ntile):
    """[ntile*128] fp32 -> [128, ntile] (per-partition bias columns)."""
    return np.ascontiguousarray(
        np.asarray(v, np.float32).reshape(ntile, 128).T)


def prep_params(params):
    f32 = lambda a: np.asarray(a, np.float32)
    out = {}

    for i, name in ((1, "ffn1"), (2, "ffn2")):
        p = params[name]
        g, b = f32(p["ln"]["g"]), f32(p["ln"]["b"])
        w1, w2 = f32(p["w1"]), f32(p["w2"])
        out[f"f{i}_w1"] = (g[:, None] * w1).astype(np.float16)
        out[f"f{i}_b1"] = _perpart(b @ w1 + f32(p["b1"]), H // 128)
        out[f"f{i}_w2"] = (0.5 * w2).astype(np.float16)
        out[f"f{i}_b2"] = _perpart(0.5 * f32(p["b2"]), D // 128)

    mp = params["mamba"]
    g, b = f32(mp["ln"]["g"]), f32(mp["ln"]["b"])
    for di, dname in ((0, "fwd"), (1, "bwd")):
        p = {k: f32(v) for k, v in mp[dname].items()}
        win = p["in_proj"]
        wxm = g[:, None] * win[:, :DI]
        bxm0 = b @ win[:, :DI]
        cw = p["conv_w"]  # [1024, 4]; causal: out_i = sum_d cw[:,3-d]*in[i-d]
        for dd in range(3):
            out[f"m{di}_wxm{dd}"] = (wxm * cw[:, 3 - dd][None, :]).astype(np.float16)
        bxm_i = np.stack(
            [bxm0 * sum(cw[:, 3 - dd] for dd in range(i + 1)) + p["conv_b"]
             for i in range(3)], 0)
        out[f"m{di}_bxm"] = np.stack(
            [_perpart(bxm_i[i], DI // 128) for i in range(3)], 0)  # [3,128,8]
        out[f"m{di}_wz"] = (g[:, None] * win[:, DI:]).astype(np.float16)
        out[f"m{di}_bz"] = _hilo(b @ win[:, DI:])
        out[f"m{di}_xproj"] = p["x_proj"].astype(np.float16)
        out[f"m{di}_dtw"] = p["dt_w"].astype(np.float16)
        out[f"m{di}_dtb"] = _hilo(p["dt_b"])
        out[f"m{di}_D"] = p["D"].astype(np.float16)[None, :]
        out[f"m{di}_wout"] = p["out_proj"].astype(np.float16)
        A = -np.exp(p["A_log"])
        expect = -(np.arange(1, DS + 1, dtype=np.float32))[None, :]
        assert np.allclose(A, np.broadcast_to(expect, A.shape),
                           rtol=1e-4, atol=1e-4), \
            "A[d,n] != -(n+1): Horner scan formulation invalid"

    p = params["conv"]
    g, b = f32(p["ln"]["g"]), f32(p["ln"]["b"])
    pw1 = f32(p["pw1_w"])
    out["c_pw1"] = (g[:, None] * pw1).astype(np.float16)
    out["c_b1"] = _perpart(b @ pw1 + f32(p["pw1_b"]), 2 * D // 128)
    dw = f32(p["dw_w"]) * f32(p["bn_g"])[:, None]
    taps = np.zeros((128, D // 128, 3, 3), np.float32)
    for l in range(3):
        for m in range(3):
            taps[:, :, l, m] = dw[:, 15 + m - l].reshape(D // 128, 128).T
    out["c_taps"] = taps
    out["c_bnb"] = _perpart(f32(p["dw_b"]) * f32(p["bn_g"]) + f32(p["bn_b"]),
                            D // 128)
    out["c_pw2"] = f32(p["pw2_w"]).astype(np.float16)
    out["c_pw2b"] = _perpart(f32(p["pw2_b"]), D // 128)

    out["lo_g"] = f32(params["ln_out"]["g"])[None, :]
    out["lo_b"] = f32(params["ln_out"]["b"])[None, :]
    return out


def bcast_ap(dram_ap, p=128):
    """DRAM [1, N] AP -> partition-broadcast [p, N] AP."""
    return bass.AP(tensor=dram_ap.tensor, offset=dram_ap.offset,
                   ap=[[0, p]] + list(dram_ap.ap[1:]))


# ---------------------------------------------------------------------------
def build_nc(BB, pp_specs):
    T = L * BB
    NBS = BB // 128
    NTT = L * NBS
    NHALF = 2 if BB >= 256 else 1
    HB = BB // NHALF
    NHB = HB // 128
    CH = 512 if T % 512 == 0 else T
    NCH = T // CH

    nc = bass.Bass(target_bir_lowering=False, trn_type="TRN2")
    dram = {}
    for l in range(L):
        dram[f"xin_{l}"] = nc.dram_tensor(f"xin_{l}", [BB, D], FP32,
                                          kind="ExternalInput")
    for name, (shape, npdt) in pp_specs.items():
        dt = FP16 if npdt == np.float16 else FP32
        dram[name] = nc.dram_tensor(name, list(shape), dt, kind="ExternalInput")
    out_d = nc.dram_tensor("out", [BB, D], FP32, kind="ExternalOutput")

    with tile.TileContext(nc) as tc, ExitStack() as ctx:
        glob = ctx.enter_context(tc.tile_pool(name="glob", bufs=1))
        resid = ctx.enter_context(tc.tile_pool(name="resid", bufs=1))
        stats = ctx.enter_context(tc.tile_pool(name="stats", bufs=NTT + 2))

        ident = glob.tile([128, 128], FP16, tag="ident")
        make_identity(nc, ident[:])
        ones2 = glob.tile([2, 128], FP16, tag="ones2")
        nc.gpsimd.memset(ones2[:], 1.0)
        eps_t = glob.tile([128, 1], FP32, tag="eps")
        nc.vector.memset(eps_t[:], EPS)

        # CoreSim implements neither Silu nor Softplus; decompose when simming.
        sim_safe = bool(int(os.environ.get("BASS_SIM_SAFE_ACT", "0")))
        simp = ctx.enter_context(tc.tile_pool(name="simp", bufs=2)) \
            if sim_safe else None

        def act_silu(out, in_, bias=0.0):
            if not sim_safe:
                nc.scalar.activation(out, in_, AF.Silu, bias=bias)
                return
            w = out.free_size()
            t1 = simp.tile([128, 2048], FP32, tag="simtmp")
            nc.scalar.activation(t1[:, 0:w], in_, AF.Identity, bias=bias)
            nc.scalar.activation(out, in_, AF.Sigmoid, bias=bias)
            nc.vector.tensor_mul(out, out, t1[:, 0:w])

        # this walrus's ACT tables have no softplus; exp and ln share one set.
        sp_pool = ctx.enter_context(tc.tile_pool(name="sp", bufs=2))

        def act_softplus(out, in_):
            w = out.free_size()
            t1 = sp_pool.tile([128, DI], FP32, tag="sptmp")
            nc.scalar.activation(t1[:, 0:w], in_, AF.Exp)
            nc.scalar.activation(out, t1[:, 0:w], AF.Ln, bias=1.0)

        x = resid.tile([128, NTT, D], FP32, tag="x")
        for l in range(L):
            for bs in range(NBS):
                nc.sync.dma_start(x[:, l * NBS + bs, :],
                                  dram[f"xin_{l}"][bs * 128:(bs + 1) * 128, :])

        # ----------------- helpers -----------------
        def ln_stats():
            res = []
            for tt in range(NTT):
                st = stats.tile([128, 6], FP32, tag="bnst")
                nc.vector.bn_stats(st[:], x[:, tt, :])
                mv = stats.tile([128, 2], FP32, tag="bnmv")
                nc.vector.bn_aggr(mv[:], st[:])
                sd = stats.tile([128, 1], FP32, tag="sd")
                nc.scalar.activation(sd[:], mv[:, 1:2], AF.Sqrt,
                                     bias=eps_t[:, 0:1])
                rs = stats.tile([128, 1], FP32, tag="rs")
                nc.vector.reciprocal(rs[:], sd[:])
                res.append((mv, rs))
            return res

        def make_xnt(pool, mus):
            """LN-apply (no gain) + transpose -> [128, 4, T] fp16 feature-major."""
            xnt = pool.tile([128, D // 128, T], FP16, tag="xnt")
            with tc.tile_pool(name="xnt_ps", bufs=2, space="PSUM") as pps, \
                 tc.tile_pool(name="xnt_tok", bufs=4) as tpool:
                for tg0 in range(0, NTT, 4):
                    gsz = min(4, NTT - tg0)
                    toks = []
                    for tb in range(gsz):
                        tt = tg0 + tb
                        mv, rs = mus[tt]
                        xtok = tpool.tile([128, D], FP16, tag="xtok")
                        nc.vector.tensor_scalar(
                            out=xtok[:], in0=x[:, tt, :], scalar1=mv[:, 0:1],
                            scalar2=rs[:, 0:1], op0=ALU.subtract, op1=ALU.mult)
                        toks.append(xtok)
                    for f in range(D // 128):
                        ps = pps.tile([128, 512], FP16, tag="ps_xnt")
                        for tb in range(gsz):
                            nc.tensor.transpose(
                                ps[:, tb * 128:(tb + 1) * 128],
                                toks[tb][:, f * 128:(f + 1) * 128], ident[:])
                        nc.vector.tensor_copy(
                            xnt[:, f, tg0 * 128:(tg0 + gsz) * 128],
                            ps[:, 0:gsz * 128])
            return xnt

        def load_w(pool, name, kt, n, tag):
            w = pool.tile([128, kt, n], FP16, tag=tag)
            nc.sync.dma_start(w[:], dram[name].rearrange("(kt p) n -> p kt n",
                                                         p=128))
            return w

        def add_residual(ptr_pool, o_t, ncols, tt_of_block):
            """o_t [128, 4, ncols] fp16 feature-major; transpose & add to x."""
            for tb in range(ncols // 128):
                ps = ptr_pool.tile([128, D], FP16, tag="ps_tr")
                for f in range(D // 128):
                    nc.tensor.transpose(ps[:, f * 128:(f + 1) * 128],
                                        o_t[:, f, tb * 128:(tb + 1) * 128],
                                        ident[:])
                tt = tt_of_block(tb)
                nc.vector.tensor_tensor(out=x[:, tt, :], in0=x[:, tt, :],
                                        in1=ps[:], op=ALU.add)

        # ----------------- FFN -----------------
        def ffn_phase(tag):
            with ExitStack() as ps:
                pool = ps.enter_context(tc.tile_pool(name=f"{tag}_s", bufs=2))
                wpool = ps.enter_context(tc.tile_pool(name=f"{tag}_w", bufs=1))
                mus = ln_stats()
                xnt = make_xnt(wpool, mus)
                w1 = load_w(wpool, f"{tag}_w1", D // 128, H, "w1")
                w2 = load_w(wpool, f"{tag}_w2", H // 128, D, "w2")
                b1 = wpool.tile([128, H // 128], FP32, tag="b1")
                nc.sync.dma_start(b1[:], dram[f"{tag}_b1"][:, :])
                b2 = wpool.tile([128, D // 128], FP32, tag="b2")
                nc.sync.dma_start(b2[:], dram[f"{tag}_b2"][:, :])

                ph_p = ps.enter_context(
                    tc.tile_pool(name=f"{tag}_ph", bufs=3, space="PSUM"))
                po_p = ps.enter_context(
                    tc.tile_pool(name=f"{tag}_po", bufs=2, space="PSUM"))
                ptr_p = ps.enter_context(
                    tc.tile_pool(name=f"{tag}_pt", bufs=2, space="PSUM"))

                for c in range(NCH):
                    cs = slice(c * CH, (c + 1) * CH)
                    h = pool.tile([128, H // 128, CH], FP16, tag="h")
                    for m in range(H // 128):
                        ph = ph_p.tile([128, CH], FP32, tag="ph")
                        for k in range(D // 128):
                            nc.tensor.matmul(ph[:],
                                             w1[:, k, m * 128:(m + 1) * 128],
                                             xnt[:, k, cs], start=(k == 0),
                                             stop=(k == D // 128 - 1))
                        act_silu(h[:, m, :], ph[:], bias=b1[:, m:m + 1])
                    o_t = pool.tile([128, D // 128, CH], FP16, tag="ot")
                    for m in range(D // 128):
                        po = po_p.tile([128, CH], FP32, tag="po")
                        for k in range(H // 128):
                            nc.tensor.matmul(po[:],
                                             w2[:, k, m * 128:(m + 1) * 128],
                                             h[:, k, :], start=(k == 0),
                                             stop=(k == H // 128 - 1))
                        nc.scalar.activation(o_t[:, m, :], po[:], AF.Identity,
                                             bias=b2[:, m:m + 1])
                    c0 = c * (CH // 128)
                    add_residual(ptr_p, o_t, CH, lambda tb: c0 + tb)

        # ----------------- Mamba -----------------
        def mamba_dir(di, xnt, ptr_p):
            lpos = (lambda i: i) if di == 0 else (lambda i: L - 1 - i)
            with ExitStack() as ds:
                wpool = ds.enter_context(tc.tile_pool(name=f"md{di}_w", bufs=1))
                wxm = [load_w(wpool, f"m{di}_wxm{dd}", D // 128, DI, f"wxm{dd}")
                       for dd in range(3)]
                wz = load_w(wpool, f"m{di}_wz", D // 128, DI, "wz")
                wout = load_w(wpool, f"m{di}_wout", DI // 128, D, "wout")
                bz = wpool.tile([2, DI], FP16, tag="bz")
                nc.sync.dma_start(bz[:], dram[f"m{di}_bz"][:, :])
                bxm = wpool.tile([128, 3, DI // 128], FP32, tag="bxm")
                nc.sync.dma_start(bxm[:],
                                  dram[f"m{di}_bxm"].rearrange("i p m -> p i m"))
                xproj = wpool.tile([128, DI // 128, 2 * RK], FP16, tag="xproj")
                nc.sync.dma_start(
                    xproj[:],
                    dram[f"m{di}_xproj"].rearrange("(kt p) n -> p kt n", p=128))
                dtw = wpool.tile([RK, DI], FP16, tag="dtw")
                nc.sync.dma_start(dtw[:], dram[f"m{di}_dtw"][:, :])
                dtb = wpool.tile([2, DI], FP16, tag="dtb")
                nc.sync.dma_start(dtb[:], dram[f"m{di}_dtb"][:, :])
                Dbc = wpool.tile([128, DI], FP16, tag="Dbc")
                nc.sync.dma_start(Dbc[:], bcast_ap(dram[f"m{di}_D"][:, :]))

                for hb in range(NHALF):
                    mamba_half(di, hb, lpos, xnt, wxm, wz, wout, bz, bxm,
                               xproj, dtw, dtb, Dbc, ptr_p)

        def mamba_half(di, hb, lpos, xnt, wxm, wz, wout, bz, bxm, xproj, dtw,
                       dtb, Dbc, ptr_p):
            HW = HB           # batch columns in this half
            NT = L * NHB      # local token tiles (scan order: t = i*NHB + bs)

            def pcols(i):     # xnt col range of scan step i (physical tokens)
                return slice(lpos(i) * BB + hb * HW, lpos(i) * BB + (hb + 1) * HW)

            def ptok(i, bs):  # physical xnt cols of one 128-token block
                base = lpos(i) * BB + hb * HW + bs * 128
                return slice(base, base + 128)

            def gtt(i, bs):   # global residual tile index
                return lpos(i) * NBS + hb * NHB + bs

            with ExitStack() as hs:
                hp = hs.enter_context(tc.tile_pool(name=f"mh{di}{hb}", bufs=1))
                tp = hs.enter_context(tc.tile_pool(name=f"mt{di}{hb}", bufs=1))
                kp = hs.enter_context(tc.tile_pool(name=f"mk{di}{hb}", bufs=2))

                # --- z (b-major) + silu ---
                sz = hp.tile([128, NT, DI], FP16, tag="sz")
                with tc.tile_pool(name=f"z{di}{hb}", bufs=3, space="PSUM") as zp:
                    for t in range(NT):
                        i, bs = t // NHB, t % NHB
                        for cc in range(DI // 512):
                            ncs = slice(cc * 512, (cc + 1) * 512)
                            pz = zp.tile([128, 512], FP32, tag="pz")
                            nc.tensor.matmul(pz[:], ones2[:, :], bz[:, ncs],
                                             start=True, stop=False)
                            for k in range(D // 128):
                                nc.tensor.matmul(pz[:], xnt[:, k, ptok(i, bs)],
                                                 wz[:, k, ncs], start=False,
                                                 stop=(k == D // 128 - 1))
                            act_silu(sz[:, t, ncs], pz[:])

                # --- xm: conv-folded in_proj (feature-major) + silu ---
                xm_t = hp.tile([128, DI // 128, L * HW], FP16, tag="xmt")
                with tc.tile_pool(name=f"px{di}{hb}", bufs=4, space="PSUM") as xp:
                    for m in range(DI // 128):
                        for i in range(L):
                            px = xp.tile([128, HW], FP32, tag="px")
                            nmm = (i + 1) * (D // 128)
                            c = 0
                            for dd in range(i + 1):
                                for k in range(D // 128):
                                    nc.tensor.matmul(
                                        px[:], wxm[dd][:, k, m * 128:(m + 1) * 128],
                                        xnt[:, k, pcols(i - dd)],
                                        start=(c == 0), stop=(c == nmm - 1))
                                    c += 1
                            act_silu(xm_t[:, m, i * HW:(i + 1) * HW], px[:],
                                     bias=bxm[:, i, m:m + 1])

                # --- xm_b: transpose to b-major ---
                xm_b = hp.tile([128, NT, DI], FP16, tag="xmb")
                with tc.tile_pool(name=f"pb{di}{hb}", bufs=2, space="PSUM") as bp:
                    for t in range(NT):
                        i, bs = t // NHB, t % NHB
                        pt = bp.tile([128, DI], FP16, tag="pxmb")
                        for cb in range(DI // 128):
                            nc.tensor.transpose(
                                pt[:, cb * 128:(cb + 1) * 128],
                                xm_t[:, cb, i * HW + bs * 128:i * HW + bs * 128 + 128],
                                ident[:])
                        nc.vector.tensor_copy(xm_b[:, t, :], pt[:])

                # --- dbc = xm_t @ x_proj ---
                dbc_r = hp.tile([RK, L * HW], FP16, tag="dbcr")
                dbc_bc = hp.tile([RK, L * HW], FP16, tag="dbcbc")
                nds = 1 if L * HW <= 512 else 2
                dcw = L * HW // nds
                assert dcw <= 512
                with tc.tile_pool(name=f"pd{di}{hb}", bufs=2, space="PSUM") as dp:
                    for c in range(nds):
                        cs = slice(c * dcw, (c + 1) * dcw)
                        pp = dp.tile([2 * RK, dcw], FP32, tag="pdbc")
                        for k in range(DI // 128):
                            nc.tensor.matmul(pp[:], xproj[:, k, :], xm_t[:, k, cs],
                                             start=(k == 0),
                                             stop=(k == DI // 128 - 1))
                        nc.vector.tensor_copy(dbc_r[:, cs], pp[0:RK, :])
                        nc.vector.tensor_copy(dbc_bc[:, cs], pp[RK:2 * RK, :])

                # --- dt_pre -> q (sigmoid(-pre)), dt (softplus(pre)) ---
                pre = hp.tile([128, NT, DI], FP16, tag="xmt")  # reuse xm_t slot
                qt = hp.tile([128, NT, DI], FP16, tag="q")
                dtt = hp.tile([128, NT, DI], FP16, tag="dt")
                with tc.tile_pool(name=f"pq{di}{hb}", bufs=4, space="PSUM") as qp:
                    for t in range(NT):
                        for cc in range(DI // 512):
                            ncs = slice(cc * 512, (cc + 1) * 512)
                            pq = qp.tile([128, 512], FP32, tag="pq")
                            nc.tensor.matmul(pq[:], ones2[:, :], dtb[:, ncs],
                                             start=True, stop=False)
                            nc.tensor.matmul(
                                pq[:], dbc_r[:, t * 128:(t + 1) * 128],
                                dtw[:, ncs], start=False, stop=True)
                            nc.vector.tensor_copy(pre[:, t, ncs], pq[:])
                for t in range(NT):
                    nc.scalar.activation(qt[:, t, :], pre[:, t, :], AF.Sigmoid,
                                         scale=-1.0)
                for t in range(NT):
                    act_softplus(dtt[:, t, :], pre[:, t, :])

                # --- B,C to b-major fp32 ---
                bc = hp.tile([128, NT, 2 * DS], FP32, tag="bc")
                with tc.tile_pool(name=f"pc{di}{hb}", bufs=2, space="PSUM") as cp:
                    for t in range(NT):
                        pb = cp.tile([128, 2 * DS], FP16, tag="pbc")
                        nc.tensor.transpose(pb[:],
                                            dbc_bc[:, t * 128:(t + 1) * 128],
                                            ident[0:2 * DS, 0:2 * DS])
                        nc.vector.tensor_copy(bc[:, t, :], pb[:])

                # --- scan ---
                y_g = hp.tile([128, NT, DI], FP16, tag="yg")
                for bs in range(NHB):
                    tl = [i * NHB + bs for i in range(L)]
                    Bv = [bc[:, tl[i], 0:DS] for i in range(L)]
                    Cv = [bc[:, tl[i], DS:2 * DS] for i in range(L)]
                    G = []
                    for i in range(L):
                        kk = kp.tile([128, DS], FP32, tag="kscr")
                        gg = kp.tile([128, 1], FP32, tag=f"gii{i}")
                        nc.vector.scalar_tensor_tensor(
                            out=kk[:], in0=Cv[i], scalar=1.0, in1=Bv[i],
                            op0=ALU.mult, op1=ALU.mult, accum_out=gg[:])
                        G.append(gg)
                    K = {}
                    for (i, j) in ((1, 0), (2, 1), (2, 0)):
                        kk = kp.tile([128, DS], FP32, tag=f"k{i}{j}")
                        nc.vector.tensor_tensor(out=kk[:], in0=Cv[i], in1=Bv[j],
                                                op=ALU.mult)
                        K[(i, j)] = kk

                    q1 = qt[:, tl[1], :]
                    q2 = qt[:, tl[2], :]
                    q20 = tp.tile([128, DI], FP16, tag="q20")
                    nc.vector.tensor_mul(q20[:], q1, q2)

                    def horner(qten, quten, kk, eng, out_tag):
                        P = tp.tile([128, DI], FP16, tag=out_tag)
                        eng.tensor_scalar_mul(P[:], qten, kk[:, DS - 1:DS])
                        for n in range(DS - 2, 0, -1):
                            eng.scalar_tensor_tensor(
                                out=P[:], in0=P[:], scalar=kk[:, n:n + 1],
                                in1=qten, op0=ALU.add, op1=ALU.mult)
                        eng.scalar_tensor_tensor(
                            out=P[:], in0=P[:], scalar=kk[:, 0:1], in1=quten,
                            op0=ALU.add, op1=ALU.mult)
                        return P

                    # u_j = dt_j * xm_j, then in-place qu_ij = q_ij * u_j
                    # (the final Horner factor carries u)
                    u0 = tp.tile([128, DI], FP16, tag="u0")
                    nc.vector.tensor_mul(u0[:], dtt[:, tl[0], :], xm_b[:, tl[0], :])
                    u1 = tp.tile([128, DI], FP16, tag="u1")
                    nc.vector.tensor_mul(u1[:], dtt[:, tl[1], :], xm_b[:, tl[1], :])
                    nc.vector.tensor_mul(u0[:], q1, u0[:])    # qu10
                    nc.vector.tensor_mul(u1[:], q2, u1[:])    # qu21
                    qu20 = tp.tile([128, DI], FP16, tag="qu20")
                    nc.gpsimd.tensor_mul(qu20[:], q2, u0[:])

                    c20 = horner(q20[:], qu20[:], K[(2, 0)], nc.vector, "c20")
                    c10 = horner(q1, u0[:], K[(1, 0)], nc.vector, "c10")
                    c21 = horner(q2, u1[:], K[(2, 1)], nc.vector, "c21")

                    for i in range(L):
                        t0 = tp.tile([128, DI], FP16, tag="t0")
                        nc.vector.scalar_tensor_tensor(
                            out=t0[:], in0=dtt[:, tl[i], :], scalar=G[i][:, 0:1],
                            in1=Dbc[:], op0=ALU.mult, op1=ALU.add)
                        ya = tp.tile([128, DI], FP16, tag="ya")
                        nc.vector.tensor_mul(ya[:], t0[:], xm_b[:, tl[i], :])
                        if i == 1:
                            nc.vector.tensor_tensor(out=ya[:], in0=ya[:],
                                                    in1=c10[:], op=ALU.add)
                        elif i == 2:
                            yb = tp.tile([128, DI], FP16, tag="q20")
                            nc.gpsimd.tensor_tensor(out=yb[:], in0=c20[:],
                                                    in1=c21[:], op=ALU.add)
                            nc.vector.tensor_tensor(out=ya[:], in0=ya[:],
                                                    in1=yb[:], op=ALU.add)
                        nc.vector.tensor_mul(y_g[:, tl[i], :], ya[:],
                                             sz[:, tl[i], :])

                # --- y_g -> feature-major ---
                ygt = hp.tile([128, DI // 128, NT * 128], FP16, tag="q")  # reuse
                with tc.tile_pool(name=f"py{di}{hb}", bufs=2, space="PSUM") as yp:
                    for cb in range(DI // 128):
                        pt = yp.tile([128, NT * 128], FP16, tag="pygt")
                        for t in range(NT):
                            nc.tensor.transpose(
                                pt[:, t * 128:(t + 1) * 128],
                                y_g[:, t, cb * 128:(cb + 1) * 128], ident[:])
                        nc.vector.tensor_copy(ygt[:, cb, :], pt[:])

                # --- out_proj + residual add ---
                nos = 1 if NT * 128 <= 512 else 2
                ocw = NT * 128 // nos
                assert ocw % 128 == 0 and ocw <= 512
                with tc.tile_pool(name=f"po{di}{hb}", bufs=2, space="PSUM") as op, \
                     tc.tile_pool(name=f"os{di}{hb}", bufs=2) as osb:
                    for c in range(nos):
                        cs = slice(c * ocw, (c + 1) * ocw)
                        o_t = osb.tile([128, D // 128, ocw], FP16, tag="mot")
                        for m in range(D // 128):
                            po = op.tile([128, ocw], FP32, tag="mpo")
                            for k in range(DI // 128):
                                nc.tensor.matmul(
                                    po[:], wout[:, k, m * 128:(m + 1) * 128],
                                    ygt[:, k, cs], start=(k == 0),
                                    stop=(k == DI // 128 - 1))
                            nc.vector.tensor_copy(o_t[:, m, :], po[:])
                        t0 = c * (ocw // 128)
                        add_residual(
                            ptr_p, o_t, ocw,
                            lambda tb: gtt((t0 + tb) // NHB, (t0 + tb) % NHB))

        def mamba_phase():
            with ExitStack() as ps:
                spool = ps.enter_context(tc.tile_pool(name="mm_s", bufs=1))
                ptr_p = ps.enter_context(
                    tc.tile_pool(name="mm_pt", bufs=2, space="PSUM"))
                mus = ln_stats()
                xnt = make_xnt(spool, mus)
                for di in range(2):
                    mamba_dir(di, xnt, ptr_p)

        # ----------------- Conv module -----------------
        def conv_phase():
            with ExitStack() as ps:
                pool = ps.enter_context(tc.tile_pool(name="cv_s", bufs=1))
                wpool = ps.enter_context(tc.tile_pool(name="cv_w", bufs=1))
                mus = ln_stats()
                xnt = make_xnt(wpool, mus)
                pw1 = load_w(wpool, "c_pw1", D // 128, 2 * D, "pw1")
                b1 = wpool.tile([128, 2 * D // 128], FP32, tag="cb1")
                nc.sync.dma_start(b1[:], dram["c_b1"][:, :])
                taps = wpool.tile([128, D // 128, 3, 3], FP32, tag="taps")
                nc.sync.dma_start(taps[:], dram["c_taps"][:, :, :, :])
                bnb = wpool.tile([128, D // 128], FP32, tag="bnb")
                nc.sync.dma_start(bnb[:], dram["c_bnb"][:, :])
                pw2 = load_w(wpool, "c_pw2", D // 128, D, "pw2")
                pw2b = wpool.tile([128, D // 128], FP32, tag="pw2b")
                nc.sync.dma_start(pw2b[:], dram["c_pw2b"][:, :])

                pa_p = ps.enter_context(
                    tc.tile_pool(name="cv_pa", bufs=2, space="PSUM"))
                ptr_p = ps.enter_context(
                    tc.tile_pool(name="cv_pt", bufs=2, space="PSUM"))

                a_t = pool.tile([128, D // 128, T], FP16, tag="a")
                s_t = pool.tile([128, D // 128, T], FP16, tag="s")
                for m in range(2 * D // 128):
                    for c in range(NCH):
                        cs = slice(c * CH, (c + 1) * CH)
                        pa = pa_p.tile([128, CH], FP32, tag="pa")
                        for k in range(D // 128):
                            nc.tensor.matmul(pa[:],
                                             pw1[:, k, m * 128:(m + 1) * 128],
                                             xnt[:, k, cs], start=(k == 0),
                                             stop=(k == D // 128 - 1))
                        if m < D // 128:
                            nc.scalar.activation(a_t[:, m, cs], pa[:],
                                                 AF.Identity, bias=b1[:, m:m + 1])
                        else:
                            nc.scalar.activation(s_t[:, m - D // 128, cs], pa[:],
                                                 AF.Sigmoid, bias=b1[:, m:m + 1])
                glu = pool.tile([128, D // 128, T], FP16, tag="glu")
                for m in range(D // 128):
                    nc.vector.tensor_mul(glu[:, m, :], a_t[:, m, :], s_t[:, m, :])
                conv = pool.tile([128, D // 128, T], FP16, tag="s")  # reuse s_t
                for m in range(D // 128):
                    for l in range(L):
                        acc = conv[:, m, l * BB:(l + 1) * BB]
                        nc.vector.tensor_scalar_mul(
                            acc, glu[:, m, 0:BB], taps[:, m, l, 0:1])
                        for mm in range(1, L):
                            nc.vector.scalar_tensor_tensor(
                                out=acc, in0=glu[:, m, mm * BB:(mm + 1) * BB],
                                scalar=taps[:, m, l, mm:mm + 1], in1=acc,
                                op0=ALU.mult, op1=ALU.add)
                cs_t = pool.tile([128, D // 128, T], FP16, tag="a")  # reuse a_t
                for m in range(D // 128):
                    act_silu(cs_t[:, m, :], conv[:, m, :], bias=bnb[:, m:m + 1])
                for c in range(NCH):
                    cs2 = slice(c * CH, (c + 1) * CH)
                    o_t = pool.tile([128, D // 128, CH], FP16, tag="cot")
                    for m in range(D // 128):
                        po = pa_p.tile([128, CH], FP32, tag="pa")
                        for k in range(D // 128):
                            nc.tensor.matmul(po[:],
                                             pw2[:, k, m * 128:(m + 1) * 128],
                                             cs_t[:, k, cs2], start=(k == 0),
                                             stop=(k == D // 128 - 1))
                        nc.scalar.activation(o_t[:, m, :], po[:], AF.Identity,
                                             bias=pw2b[:, m:m + 1])
                    c0 = c * (CH // 128)
                    add_residual(ptr_p, o_t, CH, lambda tb: c0 + tb)

        # ----------------- Output -----------------
        def out_phase():
            with ExitStack() as ps:
                pool = ps.enter_context(tc.tile_pool(name="out_s", bufs=1))
                g_bc = pool.tile([128, D], FP32, tag="g_bc")
                nc.sync.dma_start(g_bc[:], bcast_ap(dram["lo_g"][:, :]))
                b_bc = pool.tile([128, D], FP32, tag="b_bc")
                nc.sync.dma_start(b_bc[:], bcast_ap(dram["lo_b"][:, :]))
                mus = ln_stats()
                s_tiles = []
                for tt in range(NTT):
                    mv, rs = mus[tt]
                    s = pool.tile([128, D], FP32, tag=f"so{tt}")
                    nc.vector.tensor_scalar(out=s[:], in0=x[:, tt, :],
                                            scalar1=mv[:, 0:1],
                                            scalar2=rs[:, 0:1],
                                            op0=ALU.subtract, op1=ALU.mult)
                    s_tiles.append(s)
                for bs in range(NBS):
                    acc = pool.tile([128, D], FP32, tag=f"acc{bs}")
                    nc.vector.tensor_add(acc[:], s_tiles[bs][:],
                                         s_tiles[NBS + bs][:])
                    nc.vector.tensor_add(acc[:], acc[:], s_tiles[2 * NBS + bs][:])
                    nc.vector.tensor_scalar_mul(acc[:], acc[:], 1.0 / 3.0)
                    nc.vector.tensor_mul(acc[:], acc[:], g_bc[:])
                    nc.vector.tensor_add(acc[:], acc[:], b_bc[:])
                    nc.sync.dma_start(out_d[bs * 128:(bs + 1) * 128, :], acc[:])

        ffn_phase("f1")
        mamba_phase()
        conv_phase()
        ffn_phase("f2")
        out_phase()

    split_excess_waits(nc)
    return nc


# ---------------------------------------------------------------------------
_NC_CACHE = {}


def kernel(fine_messages, coarse_messages, motif_features, params):
    from concourse.bass_utils import run_bass_kernel_spmd

    B = fine_messages.shape[0]
    BB = B // NCORES
    pp = prep_params(params)
    pp_specs = {k: (v.shape, v.dtype.type) for k, v in pp.items()}

    if BB not in _NC_CACHE:
        _NC_CACHE[BB] = build_nc(BB, pp_specs)
    nc = _NC_CACHE[BB]

    xs = [np.asarray(fine_messages, np.float32),
          np.asarray(coarse_messages, np.float32),
          np.asarray(motif_features, np.float32)]
    in_maps = []
    for c in range(NCORES):
        m = dict(pp)
        for l in range(L):
            m[f"xin_{l}"] = np.ascontiguousarray(xs[l][c * BB:(c + 1) * BB])
        in_maps.append(m)

    res = run_bass_kernel_spmd(nc, in_maps, core_ids=list(range(NCORES)))
    global _LAST_RESULT
    _LAST_RESULT = res
    return np.concatenate([r["out"] for r in res.results], 0)


_LAST_RESULT = None


# revision 24
# speedup vs baseline: 2.1027x; 2.1027x over previous
"""Trainium2 Bass kernel for nn_ConBiMambaBlock (conformer macaron block with a
BiMamba mixer), pure data-parallel over 8 NeuronCores.

Per-core layout (batch shard BB rows, L=3 positions):
  - tokens l-major: token t = l*BB + b, T = 3*BB.
  - residual stream x: SBUF fp32, token-major [128 tok, 512 feat] tiles.
  - matmul land: feature-major fp16 [128 feat, tokens]; weights fp16 [K, M].
  - mamba scan: b-major fp16 [128 tok, 1024 chan]; per-token scalars are
    per-partition scalars.  The L=3 selective scan is unrolled in closed form:
      y_i = (C_i.B_i) dt_i xm_i + D xm_i + sum_{j<i} [sum_n C_i[n]B_j[n] q_ij^(n+1)] dt_j xm_j
    with q_ij = exp(-(dt_{j+1}+..+dt_i)) since A[d,n] = -(n+1) (asserted host-side).
    The 16-term power sums run as Horner chains of scalar_tensor_tensor ops.
  - LN gains/biases, the macaron 0.5, the mamba causal conv (k=4), and BN are all
    folded into weights/biases host-side.  The conformer depthwise conv (k=31)
    touches only taps 13..17 at L=3 and runs as 3-tap per-partition STT chains.
"""

import os
from contextlib import ExitStack

import numpy as np

import concourse.bass as bass
import concourse.mybir as mybir
import concourse.tile as tile
from concourse.masks import make_identity
from concourse.tile import add_dep_helper
from concourse.vector_clock import ScopedClock, VectorClock

AF = mybir.ActivationFunctionType
FP32 = mybir.dt.float32
FP16 = mybir.dt.float16
ALU = mybir.AluOpType

D = 512
DI = 1024
DS = 16
RK = 32
H = 2048
L = 3
NCORES = 8
EPS = 1e-5


# ---------------------------------------------------------------------------
# Workaround: this container's walrus rejects >2 sync-wait commands on one
# instruction; Tile's tail drain carries one wait per touched proc.  Split the
# waits across single-proc SP nops (the drain then needs none of its own).
def _patched_drain_and_barrier(self, tick_clock, wait_clock):
    nc = self.nc
    gvec = list(tick_clock.global_clock)
    n = len(gvec)
    for i, t in enumerate(gvec):
        if t <= 0:
            continue
        sub = [0] * n
        sub[i] = t
        nop_inst = nc.sync.nop()
        wait_clock.add_sem_waits(nop_inst.ins, ScopedClock({None: VectorClock(sub)}))
    nc.sync.drain()
    nc.all_engine_barrier()
    popped = nc._tile_sem_poison_stack.pop()
    assert popped is self._sem_poison
    nc.clear_and_free_semaphores(list(self.sems.allocated().values()))
    nc.all_engine_barrier()


tile.TileContext._drain_and_barrier = _patched_drain_and_barrier

MAX_WAITS = 1


def split_excess_waits(nc, maxw=MAX_WAITS):
    """Post-pass: any instruction with more than `maxw` sem waits gets the
    excess hoisted onto freshly inserted same-engine nops just before it
    (engines execute their subsequence in order, so this is equivalent)."""
    nnop = 0
    for f in nc.m.functions:
        for b in f.blocks:
            il = b.instructions
            out = []
            for inst in il:
                si = inst.sync_info
                if si is not None and si.on_wait and len(si.on_wait) > maxw:
                    waits = list(si.on_wait)
                    while len(waits) > maxw:
                        chunk, waits = waits[:maxw], waits[maxw:]
                        nop = mybir.InstNoOp(
                            name=f"I-waitsplit-{nnop}",
                            sync_info=mybir.SyncInfo(on_wait=chunk,
                                                     on_update=[]))
                        nnop += 1
                        nop.engine = inst.engine
                        nc.register_instruction(nop)
                        out.append(nop)
                    si.on_wait = waits
                out.append(inst)
            if nnop:
                b.instructions = out
    return nnop


# ---------------------------------------------------------------------------
def _hilo(b):
    """fp32 vector -> [2, N] fp16 (hi, lo) for exact rank-1 bias matmuls."""
    b = np.asarray(b, np.float32)
    hi = b.astype(np.float16)
    lo = (b - hi.astype(np.float32)).astype(np.float16)
    return np.stack([hi, lo], 0)


def _perpart(v, ntile):
    """[ntile*128] fp32 -> [128, ntile] (per-partition bias columns)."""
    return np.ascontiguousarray(
        np.asarray(v, np.float32).reshape(ntile, 128).T)


def prep_params(params):
    f32 = lambda a: np.asarray(a, np.float32)
    out = {}

    for i, name in ((1, "ffn1"), (2, "ffn2")):
        p = params[name]
        g, b = f32(p["ln"]["g"]), f32(p["ln"]["b"])
        w1, w2 = f32(p["w1"]), f32(p["w2"])
        out[f"f{i}_w1"] = (g[:, None] * w1).astype(np.float16)
        out[f"f{i}_b1"] = _perpart(b @ w1 + f32(p["b1"]), H // 128)
        out[f"f{i}_w2"] = (0.5 * w2).astype(np.float16)
        out[f"f{i}_b2"] = _perpart(0.5 * f32(p["b2"]), D // 128)

    mp = params["mamba"]
    g, b = f32(mp["ln"]["g"]), f32(mp["ln"]["b"])
    for di, dname in ((0, "fwd"), (1, "bwd")):
        p = {k: f32(v) for k, v in mp[dname].items()}
        win = p["in_proj"]
        wxm = g[:, None] * win[:, :DI]
        bxm0 = b @ win[:, :DI]
        cw = p["conv_w"]  # [1024, 4]; causal: out_i = sum_d cw[:,3-d]*in[i-d]
        for dd in range(3):
            out[f"m{di}_wxm{dd}"] = (wxm * cw[:, 3 - dd][None, :]).astype(np.float16)
        bxm_i = np.stack(
            [bxm0 * sum(cw[:, 3 - dd] for dd in range(i + 1)) + p["conv_b"]
             for i in range(3)], 0)
        out[f"m{di}_bxm"] = np.stack(
            [_perpart(bxm_i[i], DI // 128) for i in range(3)], 0)  # [3,128,8]
        out[f"m{di}_wz"] = (g[:, None] * win[:, DI:]).astype(np.float16)
        out[f"m{di}_bz"] = _hilo(b @ win[:, DI:])
        out[f"m{di}_xproj"] = p["x_proj"].astype(np.float16)
        out[f"m{di}_dtw"] = p["dt_w"].astype(np.float16)
        out[f"m{di}_dtb"] = _hilo(p["dt_b"])
        out[f"m{di}_D"] = p["D"].astype(np.float16)[None, :]
        out[f"m{di}_wout"] = p["out_proj"].astype(np.float16)
        A = -np.exp(p["A_log"])
        expect = -(np.arange(1, DS + 1, dtype=np.float32))[None, :]
        assert np.allclose(A, np.broadcast_to(expect, A.shape),
                           rtol=1e-4, atol=1e-4), \
            "A[d,n] != -(n+1): Horner scan formulation invalid"

    p = params["conv"]
    g, b = f32(p["ln"]["g"]), f32(p["ln"]["b"])
    pw1 = f32(p["pw1_w"])
    out["c_pw1"] = (g[:, None] * pw1).astype(np.float16)
    out["c_b1"] = _perpart(b @ pw1 + f32(p["pw1_b"]), 2 * D // 128)
    dw = f32(p["dw_w"]) * f32(p["bn_g"])[:, None]
    taps = np.zeros((128, D // 128, 3, 3), np.float32)
    for l in range(3):
        for m in range(3):
            taps[:, :, l, m] = dw[:, 15 + m - l].reshape(D // 128, 128).T
    out["c_taps"] = taps
    out["c_bnb"] = _perpart(f32(p["dw_b"]) * f32(p["bn_g"]) + f32(p["bn_b"]),
                            D // 128)
    out["c_pw2"] = f32(p["pw2_w"]).astype(np.float16)
    out["c_pw2b"] = _perpart(f32(p["pw2_b"]), D // 128)

    out["lo_g"] = f32(params["ln_out"]["g"])[None, :]
    out["lo_b"] = f32(params["ln_out"]["b"])[None, :]
    return out


def bcast_ap(dram_ap, p=128):
    """DRAM [1, N] AP -> partition-broadcast [p, N] AP."""
    return bass.AP(tensor=dram_ap.tensor, offset=dram_ap.offset,
                   ap=[[0, p]] + list(dram_ap.ap[1:]))


# ---------------------------------------------------------------------------
def build_nc(BB, pp_specs):
    T = L * BB
    NBS = BB // 128
    NTT = L * NBS
    NHALF = 2 if BB >= 256 else 1
    HB = BB // NHALF
    NHB = HB // 128
    CH = 512 if T % 512 == 0 else T
    NCH = T // CH

    nc = bass.Bass(target_bir_lowering=False, trn_type="TRN2")
    dram = {}
    for l in range(L):
        dram[f"xin_{l}"] = nc.dram_tensor(f"xin_{l}", [BB, D], FP32,
                                          kind="ExternalInput")
    for name, (shape, npdt) in pp_specs.items():
        dt = FP16 if npdt == np.float16 else FP32
        dram[name] = nc.dram_tensor(name, list(shape), dt, kind="ExternalInput")
    out_d = nc.dram_tensor("out", [BB, D], FP32, kind="ExternalOutput")

    # Chain ACT instructions in program order: the ACT table-set loads cost
    # ~2.7us each, and the scheduler otherwise interleaves batches of
    # different transcendentals (measured 61 set switches -> ~21).
    _orig_act = nc.scalar.activation
    _last_act = [None]

    def _chained_act(*a, **k):
        bi = _orig_act(*a, **k)
        if _last_act[0] is not None:
            add_dep_helper(bi.ins, _last_act[0], sync=False,
                           reason="act-set batching")
        _last_act[0] = bi.ins
        return bi

    nc.scalar.activation = _chained_act

    with tile.TileContext(nc) as tc, ExitStack() as ctx:
        glob = ctx.enter_context(tc.tile_pool(name="glob", bufs=1))
        resid = ctx.enter_context(tc.tile_pool(name="resid", bufs=1))
        stats = ctx.enter_context(tc.tile_pool(name="stats", bufs=NTT + 2))

        ident = glob.tile([128, 128], FP16, tag="ident")
        make_identity(nc, ident[:])
        ones2 = glob.tile([2, 128], FP16, tag="ones2")
        nc.gpsimd.memset(ones2[:], 1.0)
        eps_t = glob.tile([128, 1], FP32, tag="eps")
        nc.vector.memset(eps_t[:], EPS)

        # CoreSim implements neither Silu nor Softplus; decompose when simming.
        sim_safe = bool(int(os.environ.get("BASS_SIM_SAFE_ACT", "0")))
        simp = ctx.enter_context(tc.tile_pool(name="simp", bufs=2)) \
            if sim_safe else None

        def act_silu(out, in_, bias=0.0):
            if not sim_safe:
                nc.scalar.activation(out, in_, AF.Silu, bias=bias)
                return
            w = out.free_size()
            t1 = simp.tile([128, 2048], FP32, tag="simtmp")
            nc.scalar.activation(t1[:, 0:w], in_, AF.Identity, bias=bias)
            nc.scalar.activation(out, in_, AF.Sigmoid, bias=bias)
            nc.vector.tensor_mul(out, out, t1[:, 0:w])

        # this walrus's ACT tables have no softplus; exp and ln share one set.
        sp_pool = ctx.enter_context(tc.tile_pool(name="sp", bufs=2))

        def act_softplus(out, in_):
            w = out.free_size()
            t1 = sp_pool.tile([128, DI], FP32, tag="sptmp")
            nc.scalar.activation(t1[:, 0:w], in_, AF.Exp)
            nc.scalar.activation(out, t1[:, 0:w], AF.Ln, bias=1.0)

        x = resid.tile([128, NTT, D], FP32, tag="x")
        for l in range(L):
            for bs in range(NBS):
                nc.sync.dma_start(x[:, l * NBS + bs, :],
                                  dram[f"xin_{l}"][bs * 128:(bs + 1) * 128, :])

        # ----------------- helpers -----------------
        def ln_stats():
            res = []
            for tt in range(NTT):
                st = stats.tile([128, 6], FP32, tag="bnst")
                nc.vector.bn_stats(st[:], x[:, tt, :])
                mv = stats.tile([128, 2], FP32, tag="bnmv")
                nc.vector.bn_aggr(mv[:], st[:])
                sd = stats.tile([128, 1], FP32, tag="sd")
                nc.scalar.activation(sd[:], mv[:, 1:2], AF.Sqrt,
                                     bias=eps_t[:, 0:1])
                rs = stats.tile([128, 1], FP32, tag="rs")
                nc.vector.reciprocal(rs[:], sd[:])
                res.append((mv, rs))
            return res

        def make_xnt(pool, mus):
            """LN-apply (no gain) + transpose -> [128, 4, T] fp16 feature-major."""
            xnt = pool.tile([128, D // 128, T], FP16, tag="xnt")
            with tc.tile_pool(name="xnt_ps", bufs=2, space="PSUM") as pps, \
                 tc.tile_pool(name="xnt_tok", bufs=4) as tpool:
                for tg0 in range(0, NTT, 4):
                    gsz = min(4, NTT - tg0)
                    toks = []
                    for tb in range(gsz):
                        tt = tg0 + tb
                        mv, rs = mus[tt]
                        xtok = tpool.tile([128, D], FP16, tag="xtok")
                        nc.vector.tensor_scalar(
                            out=xtok[:], in0=x[:, tt, :], scalar1=mv[:, 0:1],
                            scalar2=rs[:, 0:1], op0=ALU.subtract, op1=ALU.mult)
                        toks.append(xtok)
                    for f in range(D // 128):
                        ps = pps.tile([128, 512], FP16, tag="ps_xnt")
                        for tb in range(gsz):
                            nc.tensor.transpose(
                                ps[:, tb * 128:(tb + 1) * 128],
                                toks[tb][:, f * 128:(f + 1) * 128], ident[:])
                        nc.vector.tensor_copy(
                            xnt[:, f, tg0 * 128:(tg0 + gsz) * 128],
                            ps[:, 0:gsz * 128])
            return xnt

        def load_w(pool, name, kt, n, tag):
            w = pool.tile([128, kt, n], FP16, tag=tag)
            nc.sync.dma_start(w[:], dram[name].rearrange("(kt p) n -> p kt n",
                                                         p=128))
            return w

        def add_residual(ptr_pool, o_t, ncols, tt_of_block):
            """o_t [128, 4, ncols] fp16 feature-major; transpose & add to x."""
            for tb in range(ncols // 128):
                ps = ptr_pool.tile([128, D], FP16, tag="ps_tr")
                for f in range(D // 128):
                    nc.tensor.transpose(ps[:, f * 128:(f + 1) * 128],
                                        o_t[:, f, tb * 128:(tb + 1) * 128],
                                        ident[:])
                tt = tt_of_block(tb)
                nc.vector.tensor_tensor(out=x[:, tt, :], in0=x[:, tt, :],
                                        in1=ps[:], op=ALU.add)

        # ----------------- FFN -----------------
        def ffn_phase(tag):
            with ExitStack() as ps:
                pool = ps.enter_context(tc.tile_pool(name=f"{tag}_s", bufs=2))
                wpool = ps.enter_context(tc.tile_pool(name=f"{tag}_w", bufs=1))
                mus = ln_stats()
                xnt = make_xnt(wpool, mus)
                w1 = load_w(wpool, f"{tag}_w1", D // 128, H, "w1")
                w2 = load_w(wpool, f"{tag}_w2", H // 128, D, "w2")
                b1 = wpool.tile([128, H // 128], FP32, tag="b1")
                nc.sync.dma_start(b1[:], dram[f"{tag}_b1"][:, :])
                b2 = wpool.tile([128, D // 128], FP32, tag="b2")
                nc.sync.dma_start(b2[:], dram[f"{tag}_b2"][:, :])

                ph_p = ps.enter_context(
                    tc.tile_pool(name=f"{tag}_ph", bufs=3, space="PSUM"))
                po_p = ps.enter_context(
                    tc.tile_pool(name=f"{tag}_po", bufs=2, space="PSUM"))
                ptr_p = ps.enter_context(
                    tc.tile_pool(name=f"{tag}_pt", bufs=2, space="PSUM"))

                for c in range(NCH):
                    cs = slice(c * CH, (c + 1) * CH)
                    h = pool.tile([128, H // 128, CH], FP16, tag="h")
                    for m in range(H // 128):
                        ph = ph_p.tile([128, CH], FP32, tag="ph")
                        for k in range(D // 128):
                            nc.tensor.matmul(ph[:],
                                             w1[:, k, m * 128:(m + 1) * 128],
                                             xnt[:, k, cs], start=(k == 0),
                                             stop=(k == D // 128 - 1))
                        act_silu(h[:, m, :], ph[:], bias=b1[:, m:m + 1])
                    o_t = pool.tile([128, D // 128, CH], FP16, tag="ot")
                    for m in range(D // 128):
                        po = po_p.tile([128, CH], FP32, tag="po")
                        for k in range(H // 128):
                            nc.tensor.matmul(po[:],
                                             w2[:, k, m * 128:(m + 1) * 128],
                                             h[:, k, :], start=(k == 0),
                                             stop=(k == H // 128 - 1))
                        nc.scalar.activation(o_t[:, m, :], po[:], AF.Identity,
                                             bias=b2[:, m:m + 1])
                    c0 = c * (CH // 128)
                    add_residual(ptr_p, o_t, CH, lambda tb: c0 + tb)

        # ----------------- Mamba -----------------
        def mamba_dir(di, xnt, ptr_p):
            lpos = (lambda i: i) if di == 0 else (lambda i: L - 1 - i)
            with ExitStack() as ds:
                wpool = ds.enter_context(tc.tile_pool(name=f"md{di}_w", bufs=1))
                wxm = [load_w(wpool, f"m{di}_wxm{dd}", D // 128, DI, f"wxm{dd}")
                       for dd in range(3)]
                wz = load_w(wpool, f"m{di}_wz", D // 128, DI, "wz")
                wout = load_w(wpool, f"m{di}_wout", DI // 128, D, "wout")
                bz = wpool.tile([2, DI], FP16, tag="bz")
                nc.sync.dma_start(bz[:], dram[f"m{di}_bz"][:, :])
                bxm = wpool.tile([128, 3, DI // 128], FP32, tag="bxm")
                nc.sync.dma_start(bxm[:],
                                  dram[f"m{di}_bxm"].rearrange("i p m -> p i m"))
                xproj = wpool.tile([128, DI // 128, 2 * RK], FP16, tag="xproj")
                nc.sync.dma_start(
                    xproj[:],
                    dram[f"m{di}_xproj"].rearrange("(kt p) n -> p kt n", p=128))
                dtw = wpool.tile([RK, DI], FP16, tag="dtw")
                nc.sync.dma_start(dtw[:], dram[f"m{di}_dtw"][:, :])
                dtb = wpool.tile([2, DI], FP16, tag="dtb")
                nc.sync.dma_start(dtb[:], dram[f"m{di}_dtb"][:, :])
                Dbc = wpool.tile([128, DI], FP16, tag="Dbc")
                nc.sync.dma_start(Dbc[:], bcast_ap(dram[f"m{di}_D"][:, :]))

                for hb in range(NHALF):
                    mamba_half(di, hb, lpos, xnt, wxm, wz, wout, bz, bxm,
                               xproj, dtw, dtb, Dbc, ptr_p)

        def mamba_half(di, hb, lpos, xnt, wxm, wz, wout, bz, bxm, xproj, dtw,
                       dtb, Dbc, ptr_p):
            HW = HB           # batch columns in this half
            NT = L * NHB      # local token tiles (scan order: t = i*NHB + bs)

            def pcols(i):     # xnt col range of scan step i (physical tokens)
                return slice(lpos(i) * BB + hb * HW, lpos(i) * BB + (hb + 1) * HW)

            def ptok(i, bs):  # physical xnt cols of one 128-token block
                base = lpos(i) * BB + hb * HW + bs * 128
                return slice(base, base + 128)

            def gtt(i, bs):   # global residual tile index
                return lpos(i) * NBS + hb * NHB + bs

            with ExitStack() as hs:
                hp = hs.enter_context(tc.tile_pool(name=f"mh{di}{hb}", bufs=1))
                tp = hs.enter_context(tc.tile_pool(name=f"mt{di}{hb}", bufs=1))
                kp = hs.enter_context(tc.tile_pool(name=f"mk{di}{hb}", bufs=2))

                # --- z (b-major) + silu ---
                sz = hp.tile([128, NT, DI], FP16, tag="sz")
                with tc.tile_pool(name=f"z{di}{hb}", bufs=3, space="PSUM") as zp:
                    for t in range(NT):
                        i, bs = t // NHB, t % NHB
                        for cc in range(DI // 512):
                            ncs = slice(cc * 512, (cc + 1) * 512)
                            pz = zp.tile([128, 512], FP32, tag="pz")
                            nc.tensor.matmul(pz[:], ones2[:, :], bz[:, ncs],
                                             start=True, stop=False)
                            for k in range(D // 128):
                                nc.tensor.matmul(pz[:], xnt[:, k, ptok(i, bs)],
                                                 wz[:, k, ncs], start=False,
                                                 stop=(k == D // 128 - 1))
                            act_silu(sz[:, t, ncs], pz[:])

                # --- xm: conv-folded in_proj (feature-major) + silu ---
                xm_t = hp.tile([128, DI // 128, L * HW], FP16, tag="xmt")
                with tc.tile_pool(name=f"px{di}{hb}", bufs=4, space="PSUM") as xp:
                    for m in range(DI // 128):
                        for i in range(L):
                            px = xp.tile([128, HW], FP32, tag="px")
                            nmm = (i + 1) * (D // 128)
                            c = 0
                            for dd in range(i + 1):
                                for k in range(D // 128):
                                    nc.tensor.matmul(
                                        px[:], wxm[dd][:, k, m * 128:(m + 1) * 128],
                                        xnt[:, k, pcols(i - dd)],
                                        start=(c == 0), stop=(c == nmm - 1))
                                    c += 1
                            act_silu(xm_t[:, m, i * HW:(i + 1) * HW], px[:],
                                     bias=bxm[:, i, m:m + 1])

                # --- xm_b: transpose to b-major ---
                xm_b = hp.tile([128, NT, DI], FP16, tag="xmb")
                with tc.tile_pool(name=f"pb{di}{hb}", bufs=2, space="PSUM") as bp:
                    for t in range(NT):
                        i, bs = t // NHB, t % NHB
                        pt = bp.tile([128, DI], FP16, tag="pxmb")
                        for cb in range(DI // 128):
                            nc.tensor.transpose(
                                pt[:, cb * 128:(cb + 1) * 128],
                                xm_t[:, cb, i * HW + bs * 128:i * HW + bs * 128 + 128],
                                ident[:])
                        nc.vector.tensor_copy(xm_b[:, t, :], pt[:])

                # --- dbc = xm_t @ x_proj ---
                dbc_r = hp.tile([RK, L * HW], FP16, tag="dbcr")
                dbc_bc = hp.tile([RK, L * HW], FP16, tag="dbcbc")
                nds = 1 if L * HW <= 512 else 2
                dcw = L * HW // nds
                assert dcw <= 512
                with tc.tile_pool(name=f"pd{di}{hb}", bufs=2, space="PSUM") as dp:
                    for c in range(nds):
                        cs = slice(c * dcw, (c + 1) * dcw)
                        pp = dp.tile([2 * RK, dcw], FP32, tag="pdbc")
                        for k in range(DI // 128):
                            nc.tensor.matmul(pp[:], xproj[:, k, :], xm_t[:, k, cs],
                                             start=(k == 0),
                                             stop=(k == DI // 128 - 1))
                        nc.vector.tensor_copy(dbc_r[:, cs], pp[0:RK, :])
                        nc.vector.tensor_copy(dbc_bc[:, cs], pp[RK:2 * RK, :])

                # --- dt_pre -> q (sigmoid(-pre)), dt (softplus(pre)) ---
                pre = hp.tile([128, NT, DI], FP16, tag="xmt")  # reuse xm_t slot
                qt = hp.tile([128, NT, DI], FP16, tag="q")
                dtt = hp.tile([128, NT, DI], FP16, tag="dt")
                with tc.tile_pool(name=f"pq{di}{hb}", bufs=4, space="PSUM") as qp:
                    for t in range(NT):
                        for cc in range(DI // 512):
                            ncs = slice(cc * 512, (cc + 1) * 512)
                            pq = qp.tile([128, 512], FP32, tag="pq")
                            nc.tensor.matmul(pq[:], ones2[:, :], dtb[:, ncs],
                                             start=True, stop=False)
                            nc.tensor.matmul(
                                pq[:], dbc_r[:, t * 128:(t + 1) * 128],
                                dtw[:, ncs], start=False, stop=True)
                            nc.vector.tensor_copy(pre[:, t, ncs], pq[:])
                for t in range(NT):
                    nc.scalar.activation(qt[:, t, :], pre[:, t, :], AF.Sigmoid,
                                         scale=-1.0)
                for t in range(NT):
                    act_softplus(dtt[:, t, :], pre[:, t, :])

                # --- B,C to b-major fp32 ---
                bc = hp.tile([128, NT, 2 * DS], FP32, tag="bc")
                with tc.tile_pool(name=f"pc{di}{hb}", bufs=2, space="PSUM") as cp:
                    for t in range(NT):
                        pb = cp.tile([128, 2 * DS], FP16, tag="pbc")
                        nc.tensor.transpose(pb[:],
                                            dbc_bc[:, t * 128:(t + 1) * 128],
                                            ident[0:2 * DS, 0:2 * DS])
                        nc.vector.tensor_copy(bc[:, t, :], pb[:])

                # --- scan ---
                y_g = hp.tile([128, NT, DI], FP16, tag="yg")
                for bs in range(NHB):
                    tl = [i * NHB + bs for i in range(L)]
                    Bv = [bc[:, tl[i], 0:DS] for i in range(L)]
                    Cv = [bc[:, tl[i], DS:2 * DS] for i in range(L)]
                    G = []
                    for i in range(L):
                        kk = kp.tile([128, DS], FP32, tag="kscr")
                        gg = kp.tile([128, 1], FP32, tag=f"gii{i}")
                        nc.vector.scalar_tensor_tensor(
                            out=kk[:], in0=Cv[i], scalar=1.0, in1=Bv[i],
                            op0=ALU.mult, op1=ALU.mult, accum_out=gg[:])
                        G.append(gg)
                    K = {}
                    for (i, j) in ((1, 0), (2, 1), (2, 0)):
                        kk = kp.tile([128, DS], FP32, tag=f"k{i}{j}")
                        nc.vector.tensor_tensor(out=kk[:], in0=Cv[i], in1=Bv[j],
                                                op=ALU.mult)
                        K[(i, j)] = kk

                    q1 = qt[:, tl[1], :]
                    q2 = qt[:, tl[2], :]
                    q20 = tp.tile([128, DI], FP16, tag="q20")
                    nc.gpsimd.tensor_mul(q20[:], q1, q2)

                    def horner(qten, quten, kk, eng, out_tag):
                        # STT has no 2x DVE uop (1303 ns/step); a TS(4x)+TT(2x)
                        # pair for the same step is 1188 ns.
                        P = tp.tile([128, DI], FP16, tag=out_tag)
                        eng.tensor_scalar_mul(P[:], qten, kk[:, DS - 1:DS])
                        for n in range(DS - 2, 0, -1):
                            eng.tensor_scalar_add(P[:], P[:], kk[:, n:n + 1])
                            eng.tensor_tensor(out=P[:], in0=P[:], in1=qten,
                                              op=ALU.mult)
                        eng.tensor_scalar_add(P[:], P[:], kk[:, 0:1])
                        eng.tensor_tensor(out=P[:], in0=P[:], in1=quten,
                                          op=ALU.mult)
                        return P

                    # u_j = dt_j * xm_j, then in-place qu_ij = q_ij * u_j
                    # (the final Horner factor carries u)
                    u0 = tp.tile([128, DI], FP16, tag="u0")
                    nc.vector.tensor_mul(u0[:], dtt[:, tl[0], :], xm_b[:, tl[0], :])
                    u1 = tp.tile([128, DI], FP16, tag="u1")
                    nc.vector.tensor_mul(u1[:], dtt[:, tl[1], :], xm_b[:, tl[1], :])
                    nc.gpsimd.tensor_mul(u0[:], q1, u0[:])    # qu10
                    nc.gpsimd.tensor_mul(u1[:], q2, u1[:])    # qu21
                    qu20 = tp.tile([128, DI], FP16, tag="qu20")
                    nc.gpsimd.tensor_mul(qu20[:], q2, u0[:])

                    c20 = horner(q20[:], qu20[:], K[(2, 0)], nc.vector, "c20")
                    c10 = horner(q1, u0[:], K[(1, 0)], nc.vector, "c10")
                    c21 = horner(q2, u1[:], K[(2, 1)], nc.vector, "c21")

                    for i in range(L):
                        t0 = tp.tile([128, DI], FP16, tag="t0")
                        nc.vector.scalar_tensor_tensor(
                            out=t0[:], in0=dtt[:, tl[i], :], scalar=G[i][:, 0:1],
                            in1=Dbc[:], op0=ALU.mult, op1=ALU.add)
                        ya = tp.tile([128, DI], FP16, tag="ya")
                        nc.vector.tensor_mul(ya[:], t0[:], xm_b[:, tl[i], :])
                        if i == 1:
                            nc.gpsimd.tensor_tensor(out=ya[:], in0=ya[:],
                                                    in1=c10[:], op=ALU.add)
                        elif i == 2:
                            yb = tp.tile([128, DI], FP16, tag="q20")
                            nc.gpsimd.tensor_tensor(out=yb[:], in0=c20[:],
                                                    in1=c21[:], op=ALU.add)
                            nc.gpsimd.tensor_tensor(out=ya[:], in0=ya[:],
                                                    in1=yb[:], op=ALU.add)
                        nc.gpsimd.tensor_mul(y_g[:, tl[i], :], ya[:],
                                             sz[:, tl[i], :])

                # --- y_g -> feature-major ---
                ygt = hp.tile([128, DI // 128, NT * 128], FP16, tag="q")  # reuse
                with tc.tile_pool(name=f"py{di}{hb}", bufs=2, space="PSUM") as yp:
                    for cb in range(DI // 128):
                        pt = yp.tile([128, NT * 128], FP16, tag="pygt")
                        for t in range(NT):
                            nc.tensor.transpose(
                                pt[:, t * 128:(t + 1) * 128],
                                y_g[:, t, cb * 128:(cb + 1) * 128], ident[:])
                        nc.vector.tensor_copy(ygt[:, cb, :], pt[:])

                # --- out_proj + residual add ---
                nos = 1 if NT * 128 <= 512 else 2
                ocw = NT * 128 // nos
                assert ocw % 128 == 0 and ocw <= 512
                with tc.tile_pool(name=f"po{di}{hb}", bufs=2, space="PSUM") as op, \
                     tc.tile_pool(name=f"os{di}{hb}", bufs=2) as osb:
                    for c in range(nos):
                        cs = slice(c * ocw, (c + 1) * ocw)
                        o_t = osb.tile([128, D // 128, ocw], FP16, tag="mot")
                        for m in range(D // 128):
                            po = op.tile([128, ocw], FP32, tag="mpo")
                            for k in range(DI // 128):
                                nc.tensor.matmul(
                                    po[:], wout[:, k, m * 128:(m + 1) * 128],
                                    ygt[:, k, cs], start=(k == 0),
                                    stop=(k == DI // 128 - 1))
                            nc.vector.tensor_copy(o_t[:, m, :], po[:])
                        t0 = c * (ocw // 128)
                        add_residual(
                            ptr_p, o_t, ocw,
                            lambda tb: gtt((t0 + tb) // NHB, (t0 + tb) % NHB))

        def mamba_phase():
            with ExitStack() as ps:
                spool = ps.enter_context(tc.tile_pool(name="mm_s", bufs=1))
                ptr_p = ps.enter_context(
                    tc.tile_pool(name="mm_pt", bufs=2, space="PSUM"))
                mus = ln_stats()
                xnt = make_xnt(spool, mus)
                for di in range(2):
                    mamba_dir(di, xnt, ptr_p)

        # ----------------- Conv module -----------------
        def conv_phase():
            with ExitStack() as ps:
                pool = ps.enter_context(tc.tile_pool(name="cv_s", bufs=1))
                wpool = ps.enter_context(tc.tile_pool(name="cv_w", bufs=1))
                mus = ln_stats()
                xnt = make_xnt(wpool, mus)
                pw1 = load_w(wpool, "c_pw1", D // 128, 2 * D, "pw1")
                b1 = wpool.tile([128, 2 * D // 128], FP32, tag="cb1")
                nc.sync.dma_start(b1[:], dram["c_b1"][:, :])
                taps = wpool.tile([128, D // 128, 3, 3], FP32, tag="taps")
                nc.sync.dma_start(taps[:], dram["c_taps"][:, :, :, :])
                bnb = wpool.tile([128, D // 128], FP32, tag="bnb")
                nc.sync.dma_start(bnb[:], dram["c_bnb"][:, :])
                pw2 = load_w(wpool, "c_pw2", D // 128, D, "pw2")
                pw2b = wpool.tile([128, D // 128], FP32, tag="pw2b")
                nc.sync.dma_start(pw2b[:], dram["c_pw2b"][:, :])

                pa_p = ps.enter_context(
                    tc.tile_pool(name="cv_pa", bufs=2, space="PSUM"))
                ptr_p = ps.enter_context(
                    tc.tile_pool(name="cv_pt", bufs=2, space="PSUM"))

                a_t = pool.tile([128, D // 128, T], FP16, tag="a")
                s_t = pool.tile([128, D // 128, T], FP16, tag="s")
                for m in range(2 * D // 128):
                    for c in range(NCH):
                        cs = slice(c * CH, (c + 1) * CH)
                        pa = pa_p.tile([128, CH], FP32, tag="pa")
                        for k in range(D // 128):
                            nc.tensor.matmul(pa[:],
                                             pw1[:, k, m * 128:(m + 1) * 128],
                                             xnt[:, k, cs], start=(k == 0),
                                             stop=(k == D // 128 - 1))
                        if m < D // 128:
                            nc.scalar.activation(a_t[:, m, cs], pa[:],
                                                 AF.Identity, bias=b1[:, m:m + 1])
                        else:
                            nc.scalar.activation(s_t[:, m - D // 128, cs], pa[:],
                                                 AF.Sigmoid, bias=b1[:, m:m + 1])
                glu = pool.tile([128, D // 128, T], FP16, tag="glu")
                for m in range(D // 128):
                    nc.gpsimd.tensor_mul(glu[:, m, :], a_t[:, m, :], s_t[:, m, :])
                conv = pool.tile([128, D // 128, T], FP16, tag="s")  # reuse s_t
                for m in range(D // 128):
                    for l in range(L):
                        acc = conv[:, m, l * BB:(l + 1) * BB]
                        nc.vector.tensor_scalar_mul(
                            acc, glu[:, m, 0:BB], taps[:, m, l, 0:1])
                        for mm in range(1, L):
                            nc.vector.scalar_tensor_tensor(
                                out=acc, in0=glu[:, m, mm * BB:(mm + 1) * BB],
                                scalar=taps[:, m, l, mm:mm + 1], in1=acc,
                                op0=ALU.mult, op1=ALU.add)
                cs_t = pool.tile([128, D // 128, T], FP16, tag="a")  # reuse a_t
                for m in range(D // 128):
                    act_silu(cs_t[:, m, :], conv[:, m, :], bias=bnb[:, m:m + 1])
                for c in range(NCH):
                    cs2 = slice(c * CH, (c + 1) * CH)
                    o_t = pool.tile([128, D // 128, CH], FP16, tag="cot")
                    for m in range(D // 128):
                        po = pa_p.tile([128, CH], FP32, tag="pa")
                        for k in range(D // 128):
                            nc.tensor.matmul(po[:],
                                             pw2[:, k, m * 128:(m + 1) * 128],
                                             cs_t[:, k, cs2], start=(k == 0),
                                             stop=(k == D // 128 - 1))
                        nc.scalar.activation(o_t[:, m, :], po[:], AF.Identity,
                                             bias=pw2b[:, m:m + 1])
                    c0 = c * (CH // 128)
                    add_residual(ptr_p, o_t, CH, lambda tb: c0 + tb)

        # ----------------- Output -----------------
        def out_phase():
            with ExitStack() as ps:
                pool = ps.enter_context(tc.tile_pool(name="out_s", bufs=1))
                g_bc = pool.tile([128, D], FP32, tag="g_bc")
                nc.sync.dma_start(g_bc[:], bcast_ap(dram["lo_g"][:, :]))
                b_bc = pool.tile([128, D], FP32, tag="b_bc")
                nc.sync.dma_start(b_bc[:], bcast_ap(dram["lo_b"][:, :]))
                mus = ln_stats()
                s_tiles = []
                for tt in range(NTT):
                    mv, rs = mus[tt]
                    s = pool.tile([128, D], FP32, tag=f"so{tt}")
                    nc.vector.tensor_scalar(out=s[:], in0=x[:, tt, :],
                                            scalar1=mv[:, 0:1],
                                            scalar2=rs[:, 0:1],
                                            op0=ALU.subtract, op1=ALU.mult)
                    s_tiles.append(s)
                for bs in range(NBS):
                    acc = pool.tile([128, D], FP32, tag=f"acc{bs}")
                    nc.vector.tensor_add(acc[:], s_tiles[bs][:],
                                         s_tiles[NBS + bs][:])
                    nc.vector.tensor_add(acc[:], acc[:], s_tiles[2 * NBS + bs][:])
                    nc.vector.tensor_scalar_mul(acc[:], acc[:], 1.0 / 3.0)
                    nc.vector.tensor_mul(acc[:], acc[:], g_bc[:])
                    nc.vector.tensor_add(acc[:], acc[:], b_bc[:])
                    nc.sync.dma_start(out_d[bs * 128:(bs + 1) * 128, :], acc[:])

        ffn_phase("f1")
        mamba_phase()
        conv_phase()
        ffn_phase("f2")
        out_phase()

    split_excess_waits(nc)
    return nc


# ---------------------------------------------------------------------------
_NC_CACHE = {}


def kernel(fine_messages, coarse_messages, motif_features, params):
    from concourse.bass_utils import run_bass_kernel_spmd

    B = fine_messages.shape[0]
    BB = B // NCORES
    pp = prep_params(params)
    pp_specs = {k: (v.shape, v.dtype.type) for k, v in pp.items()}

    if BB not in _NC_CACHE:
        _NC_CACHE[BB] = build_nc(BB, pp_specs)
    nc = _NC_CACHE[BB]

    xs = [np.asarray(fine_messages, np.float32),
          np.asarray(coarse_messages, np.float32),
          np.asarray(motif_features, np.float32)]
    in_maps = []
    for c in range(NCORES):
        m = dict(pp)
        for l in range(L):
            m[f"xin_{l}"] = np.ascontiguousarray(xs[l][c * BB:(c + 1) * BB])
        in_maps.append(m)

    res = run_bass_kernel_spmd(nc, in_maps, core_ids=list(range(NCORES)))
    global _LAST_RESULT
    _LAST_RESULT = res
    return np.concatenate([r["out"] for r in res.results], 0)


_LAST_RESULT = None
